# revision 7
# baseline (speedup 1.0000x reference)
"""Guided channel-wise 3x3 conv (per-pixel weights) on 8 Trainium2 cores.

out[b,c,h,w] = sum_{dh,dw in {-1,0,1}} input[b,c,h+dh,w+dw] * weights[b,c,k(dh,dw),h,w]
with SAME zero padding.  Shapes: input (8,64,128,128) f32,
weights (8,64,9,128,128) f32 -> out (8,64,128,128) f32.

Sharding: pure data parallelism, one batch sample per NeuronCore (B=8 cores).

v2 strategy (vs the 81.5us all-fp16 HWDGE baseline):
 - Weights are stored int8 in DRAM (per-PARTITION symmetric scale s_p =
   max|w_p|; the s_p/127 factor is folded into the fp16 input windows
   host-side, so the device kernel needs no rescale anywhere).  Measured
   end-to-end rel err ~1.1e-2 < 2e-2 gate.
 - Weight loads go through the gpsimd software-DGE queue as CAST DMAs
   (int8 DRAM -> fp16 SBUF).  Measured: 420 B/ns dst-side on the 16 DMA
   engines (vs 326 B/ns for the HWDGE fp16 path) and HBM src bytes halved.
   DMA becomes dst-(engine)-bound at ~23 MB dst -> ~55us.
 - DVE does the per-tap products in-place in the fp16 weight buffer
   (0.578 ns/elem windowed, measured quiet) -> ~43us, overlapped.
 - PE reduces the 9 taps as identity-matmuls accumulating in PSUM (f32),
   ACT drains PSUM -> fp16 out buffer, sync-engine HWDGE stores.
 - gpsimd cast DMAs were measured NOT to contend with DVE products.

Per-core layout: 128 SBUF partitions = (half, c) with p = half*64 + c; each
partition holds one 64-row half of one channel plane.  The input is pre-padded
on the host into the per-partition SBUF layout (66 x 130 fp16, zero border).

DMA completion on a DGE queue is NOT in-order across the 16 engines, so every
consumer waits on a semaphore only its own producer DMA increments.  Weight
DRAM layout is (partition, chunk, tap, row, col) so chunk loads are one
contiguous cast DMA each; the last chunk streams as 3+3+1+1+1 tap groups so
only a 1024-elem product remains serial behind the very last load.
"""

import numpy as np

from concourse import bass, mybir
from concourse.bass_utils import run_bass_kernel_spmd

B, CI, H, W = 8, 64, 128, 128
K = 9
HH = H // 2  # rows per half-plane (64)
PR = HH + 2  # padded rows per partition (66)
PC = W + 2  # padded cols (130)
NP = 128  # SBUF partitions
FP = HH * W  # free elems per partition of one output half-plane (8192)

C = 8  # row-chunks per half-plane
CR = HH // C  # rows per chunk (8)
CH = CR * W  # elems per chunk per partition (1024)
BLK = 512  # matmul moving-dim block (= one PSUM bank of f32)
NB = CH // BLK  # matmul blocks per chunk (2)
NPS = 4  # PSUM chunk buffers (4 x 2 banks = all 8)

WSZ = C * K * CH  # weight elems per partition (73728)

# Weight DMA groups per chunk: (klo, khi) tap range, loaded as one cast DMA.
# Full chunks load all 9 taps at once (one ~18KB-dst descriptor run per
# partition); the last chunk is split 3/3/1/1/1 so the final serial product
# is small.  Product groups are tap-ranges within one dh row (dh = klo//3).
FULL_LOAD = [(0, 9)]
LAST_LOAD = [(0, 3), (3, 6), (6, 7), (7, 8), (8, 9)]
LOADS = [FULL_LOAD] * (C - 1) + [LAST_LOAD]
FULL_PROD = [(0, 3, 0), (3, 6, 0), (6, 9, 0)]  # (klo, khi, load_group_idx)
LAST_PROD = [(0, 3, 0), (3, 6, 1), (6, 7, 2), (7, 8, 3), (8, 9, 4)]
PRODS = [FULL_PROD] * (C - 1) + [LAST_PROD]

F16 = mybir.dt.float16
F32 = mybir.dt.float32
I8 = mybir.dt.int8


IN_SPLIT = 34 * PC  # input piece 0: padded rows [0,34) -> covers chunks 0-3


def build_bass():
    nc = bass.Bass()
    ident_d = nc.declare_dram_parameter("ident", [NP, NP], F16, isOutput=False)
    inp_d = nc.declare_dram_parameter("input", [NP, PR * PC], F16, isOutput=False)
    wts_d = nc.declare_dram_parameter("weights", [NP, WSZ], I8, isOutput=False)
    # chunk 0 duplicated as fp16 (same integer values as the int8): loaded on
    # the sync HWDGE queue, which starts ~4us before the gpsimd sw queue.
    w0_d = nc.declare_dram_parameter("w0", [NP, K * CH], F16, isOutput=False)
    out_d = nc.declare_dram_parameter("out", [NP, FP], F16, isOutput=True)

    from contextlib import ExitStack

    with ExitStack() as ctx:
        ident = ctx.enter_context(nc.sbuf_tensor("ident_s", [NP, NP], F16))
        in_pad = ctx.enter_context(nc.sbuf_tensor("in_pad", [NP, PR * PC], F16))
        wt = ctx.enter_context(nc.sbuf_tensor("wt", [NP, WSZ], F16))
        out_t = ctx.enter_context(nc.sbuf_tensor("out_t", [NP, FP], F16))
        ps = [
            ctx.enter_context(nc.psum_tensor(f"ps{j}", [NP, CH], F32))
            for j in range(NPS)
        ]
        block = ctx.enter_context(nc.Block())
        isem = ctx.enter_context(nc.semaphore("isem"))
        nsem = ctx.enter_context(nc.semaphore("nsem"))
        wsem = [
            [
                ctx.enter_context(nc.semaphore(f"wsem_{c}_{g}"))
                for g in range(len(LOADS[c]))
            ]
            for c in range(C)
        ]
        dve_sem = ctx.enter_context(nc.semaphore("dve_sem"))
        pe_sem = ctx.enter_context(nc.semaphore("pe_sem"))
        act_sem = ctx.enter_context(nc.semaphore("act_sem"))
        st_sem = ctx.enter_context(nc.semaphore("st_sem"))

        # weight region for (chunk c, tap k): contiguous CH elems
        def woff(c, k):
            return c * (K * CH) + k * CH

        def custom_ap(base, pattern, offset):
            a = base.copy()
            a.ap[:] = pattern
            a.offset = offset
            return a

        N_ST = 4  # batched stores: chunks 0-3, 4-6, then 2 half-bank pieces

        @block.sync
        def _(sync):
            # input piece 0 + fp16 chunk 0 first: products can start before
            # the gpsimd cast stream has delivered anything
            sync.dma_start(out=in_pad[:, :IN_SPLIT], in_=inp_d[:, :IN_SPLIT]).then_inc(
                nsem, 16
            )
            sync.dma_start(out=wt[:, : K * CH], in_=w0_d[:]).then_inc(wsem[0][0], 16)
            sync.dma_start(
                out=in_pad[:, IN_SPLIT:], in_=inp_d[:, IN_SPLIT:]
            ).then_inc(nsem, 16)
            sync.dma_start(out=ident[:], in_=ident_d[:]).then_inc(isem, 16)
            # Batched stores (few, large descriptors -- small ones starve the
            # shared DMA engines).  The last chunk stores per PSUM bank.
            sync.wait_ge(act_sem, 4)
            sync.dma_start(out=out_d[:, : 4 * CH], in_=out_t[:, : 4 * CH]).then_inc(
                st_sem, 16
            )
            sync.wait_ge(act_sem, C - 1)
            sync.dma_start(
                out=out_d[:, 4 * CH : 7 * CH], in_=out_t[:, 4 * CH : 7 * CH]
            ).then_inc(st_sem, 16)
            for b in range(NB):
                lo = (C - 1) * CH + b * BLK
                sync.wait_ge(act_sem, C + b)
                sync.dma_start(
                    out=out_d[:, lo : lo + BLK], in_=out_t[:, lo : lo + BLK]
                ).then_inc(st_sem, 16)
            sync.wait_ge(st_sem, 16 * N_ST)

        @block.gpsimd
        def _(gpsimd):
            # weight streaming: software-DGE cast DMAs int8 -> fp16
            # (chunk 0 arrives as fp16 on the sync queue instead)
            for c in range(1, C):
                for g, (klo, khi) in enumerate(LOADS[c]):
                    lo, hi = woff(c, klo), woff(c, khi - 1) + CH
                    gpsimd.dma_start(
                        out=wt[:, lo:hi], in_=wts_d[:, lo:hi]
                    ).then_inc(wsem[c][g], 16)

        @block.vector
        def _(vector):
            # products, in-place into the cast weight regions; each product
            # group is ONE fused tensor_tensor: the group's taps become a
            # third free dim (weight regions stride CH apart; the matching
            # input windows stride 1 apart in dw)
            vector.wait_ge(nsem, 16)
            for c in range(C):
                if c == 4:
                    # chunks 4+ read padded rows >= 34 (input piece 1)
                    vector.wait_ge(nsem, 32)
                r0 = c * CR
                last_g = -1
                for klo, khi, g in PRODS[c]:
                    nt = khi - klo
                    dh, dw = klo // 3, klo % 3
                    if g != last_g:
                        vector.wait_ge(wsem[c][g], 16)
                        last_g = g
                    wv = custom_ap(
                        wt[:],
                        [[WSZ, NP], [CH, nt], [W, CR], [1, W]],
                        woff(c, klo),
                    )
                    iv = custom_ap(
                        in_pad[:],
                        [[PR * PC, NP], [1, nt], [PC, CR], [1, W]],
                        (dh + r0) * PC + dw,
                    )
                    vector.tensor_tensor(
                        out=wv, in0=wv, in1=iv, op=mybir.AluOpType.mult
                    ).then_inc(dve_sem, 1)

        # dve_sem value after the product covering tap k of chunk c
        gbase = [sum(len(PRODS[cc]) for cc in range(c)) for c in range(C)]

        def dve_count(c, k):
            for g, (klo, khi, _lg) in enumerate(PRODS[c]):
                if klo <= k < khi:
                    return gbase[c] + g + 1
            raise AssertionError

        @block.tensor
        def _(tensor):
            # 9-tap reduction: psum[chunk] += I @ p_k (f32 accumulation)
            tensor.wait_ge(isem, 16)
            last_wait = 0
            for c in range(C):
                if c >= NPS:
                    tensor.wait_ge(act_sem, c - NPS + 1)
                pb = ps[c % NPS]
                for k in range(K):
                    for b in range(NB):
                        if dve_count(c, k) > last_wait:
                            last_wait = dve_count(c, k)
                            tensor.wait_ge(dve_sem, last_wait)
                        inst = tensor.matmul(
                            out=pb[:, b * BLK : (b + 1) * BLK],
                            lhsT=ident[:],
                            rhs=wt[:, woff(c, k) + b * BLK : woff(c, k) + (b + 1) * BLK],
                            start=(k == 0),
                            stop=(k == K - 1),
                            skip_group_check=True,
                        )
                        if k == K - 1 and (c == C - 1 or b == NB - 1):
                            # last chunk: per-bank completion for a finer tail
                            inst.then_inc(pe_sem, 1)

        @block.scalar
        def _(scalar):
            # drain PSUM -> fp16 out buffer
            for c in range(C - 1):
                scalar.wait_ge(pe_sem, c + 1)
                scalar.activation(
                    out=out_t[:, c * CH : (c + 1) * CH],
                    in_=ps[c % NPS][:],
                    func=mybir.ActivationFunctionType.Copy,
                ).then_inc(act_sem, 1)
            # last chunk: per-bank copy for a finer tail
            c = C - 1
            for b in range(NB):
                lo = c * CH + b * BLK
                scalar.wait_ge(pe_sem, c + b + 1)
                scalar.activation(
                    out=out_t[:, lo : lo + BLK],
                    in_=ps[c % NPS][:, b * BLK : (b + 1) * BLK],
                    func=mybir.ActivationFunctionType.Copy,
                ).then_inc(act_sem, 1)

    return nc


def _prep_weights(w):
    """(64,9,128,128) f32 -> int8 [128, C*K*CH] + per-partition scales [128].

    partition p = half*64 + channel; free = (row-chunk, tap, row-in-chunk, col)
    so each (chunk, tap-range) is one contiguous cast DMA per partition.
    """
    wr = (
        w.reshape(CI, K, 2, C, CR, W)
        .transpose(2, 0, 3, 1, 4, 5)
        .reshape(NP, WSZ)
        .astype(np.float32)
    )
    s = np.abs(wr).max(axis=1)  # [128]
    s = np.maximum(s, 1e-30)
    wi8 = np.rint(wr * (127.0 / s[:, None])).astype(np.int8)
    return np.ascontiguousarray(wi8), s


def _prep_input(x, s):
    """(64,128,128) f32 -> (128, 66*130) fp16 padded layout, scaled by s_p/127."""
    pad = np.zeros((CI, H + 2, W + 2), dtype=np.float32)
    pad[:, 1 : H + 1, 1 : W + 1] = x
    win = np.stack([pad[:, 0:PR, :], pad[:, HH : HH + PR, :]], axis=0)
    win = win.reshape(NP, PR * PC) * (s[:, None] / 127.0)
    return np.ascontiguousarray(win.astype(np.float16))


def _unprep_out(o):
    """(128, 64*128) fp16 -> (64,128,128) f32."""
    return np.ascontiguousarray(
        np.asarray(o)
        .astype(np.float32)
        .reshape(2, CI, HH, W)
        .transpose(1, 0, 2, 3)
        .reshape(CI, H, W)
    )


_IDENT = np.eye(NP, dtype=np.float16)

_NC = None


def _get_nc():
    global _NC
    if _NC is None:
        _NC = build_bass()
    return _NC


def make_in_maps(input, weights):
    input = np.asarray(input, dtype=np.float32)
    weights = np.asarray(weights, dtype=np.float32)
    maps = []
    for b in range(B):
        wi8, s = _prep_weights(weights[b])
        maps.append(
            {
                "ident": _IDENT,
                "input": _prep_input(input[b], s),
                "weights": wi8,
                "w0": np.ascontiguousarray(wi8[:, : K * CH].astype(np.float16)),
            }
        )
    return maps


def kernel(input, weights):
    nc = _get_nc()
    in_maps = make_in_maps(input, weights)
    res = run_bass_kernel_spmd(nc, in_maps, list(range(B)))
    return np.stack([_unprep_out(res.results[b]["out"]) for b in range(B)], axis=0)
